# revision 4
# baseline (speedup 1.0000x reference)
"""Multi-head attention + layernorm Bass kernel for Trainium2 (8 NeuronCores).

Sharding v2: tensor-parallel over heads. Core c owns global heads (2c, 2c+1)
for BOTH batches: it computes Q/K/V projections for just its 128 feature
columns, attention for its head pair over all 4096 tokens, then an 8-way
AllToAll exchanges attention outputs so core c ends up holding all 1024
features for token window c (batch c//4, queries (c%4)*512..+512), where it
runs the output projection + layernorm. Total matmul work per core is ~35%
less than the v1 batch x query-quarter sharding (which recomputed full K/V
projections on every core of a batch group).

Layouts:
  - xT [E, T=4096] bf16 (both batches concatenated along tokens), loaded
    per 512-token block so phase-1 matmuls start before the full load lands
  - kT/qT in [d, token] (128 partitions = 2 heads x 64 d)
  - scores TRANSPOSED per key chunk: sc[key, qA|qB] via two quadrant
    matmuls (heads A/B in array row halves); exp output feeds AV directly
  - V in [key, hA 0:64 |ones| hB 65:129 |ones| pad] so each AV matmul
    also emits that head's softmax rowsum in psum row 64
  - softmax skips max-subtraction: |scores|/8 <= ~9 for this distribution
  - AllToAll buffers [8, 128, 512] bf16: shard j = my pair's normalized y
    for token window j; received slot i = pair i's y for my window
"""

import numpy as np
import ml_dtypes

import concourse.bass as bass
import concourse.mybir as mybir
import concourse.tile as tile
from concourse.bass_utils import run_bass_kernel_spmd

BF16 = ml_dtypes.bfloat16
F32 = mybir.dt.float32
B16 = mybir.dt.bfloat16

B, S, E, H, D = 2, 2048, 1024, 16, 64
T = B * S         # 4096 tokens across both batches
NCORES = 8
NCE = E // 128    # 8 contraction chunks over E
NSB = T // 512    # 8 token blocks (also the 8 attention blocks / A2A shards)
NKC = T // 128    # 32 key chunks
PW = 2 * D        # 128 features per head pair
VW = D + 1        # 65: head's V columns + ones column
VROW = 200        # vA[0:64] ones[64] vB[65:129] ones[129] pad[130:200]

_CACHE = {}


def _bcast_ap(handle, n):
    """AP reading a [n]-element DRAM vector broadcast across 128 partitions."""
    ap = handle[:]
    return bass.AP(tensor=ap.tensor, offset=ap.offset, ap=[[0, 128], [1, n]])


def _rep_ap(sbuf_slice, rep):
    """Repeat a [128, n] SBUF slice `rep` times along a middle axis."""
    ap = sbuf_slice
    return bass.AP(tensor=ap.tensor, offset=ap.offset,
                   ap=[ap.ap[0], [0, rep], ap.ap[1]])


def _split_drain_waits(nc):
    """This walrus build encodes at most ONE sem wait per instruction;
    Tile emits several on some (drain, multi-dep compute/DMA). Merge waits
    on the same semaphore (sem-ge-imm: max value implies the rest), then
    hoist all but the last onto standalone EventSemaphore instructions
    placed just before, in the same engine's stream."""
    n = 0
    for f in nc.m.functions:
        for blk in f.blocks:
            new_insts = []
            for inst in blk.instructions:
                si = getattr(inst, "sync_info", None)
                if si is not None and len(si.on_wait) > 1:
                    merged = {}
                    rest = []
                    for w in si.on_wait:
                        if w.wait_mode == "sem-ge-imm":
                            k = w.id
                            if k not in merged or merged[k].wait_value < w.wait_value:
                                merged[k] = w
                        else:
                            rest.append(w)
                    waits = rest + list(merged.values())
                    for w in waits[:-1]:
                        n += 1
                        ev = mybir.InstEventSemaphore(
                            name=f"I-splitwait-{n}",
                            ins=[], outs=[],
                            sync_info=mybir.SyncInfo(on_wait=[w], on_update=[]),
                        )
                        ev.engine = inst.engine
                        new_insts.append(ev)
                    inst.sync_info = mybir.SyncInfo(
                        on_wait=[waits[-1]], on_update=list(si.on_update))
                new_insts.append(inst)
            blk.instructions[:] = new_insts
    return n


def _build_program():
    nc = bass.Bass()
    AF = mybir.ActivationFunctionType
    OP = mybir.AluOpType

    xT = nc.declare_dram_parameter("xT", [E, T], B16, isOutput=False)
    # per-core 128-column slices of Wq/Wk/Wv, host-shuffled to [p, c, d]
    wq_d = nc.declare_dram_parameter("wq", [128, NCE, 128], B16, isOutput=False)
    wk_d = nc.declare_dram_parameter("wk", [128, NCE, 128], B16, isOutput=False)
    wv_d = nc.declare_dram_parameter("wv", [128, NCE, 128], B16, isOutput=False)
    wp_d = nc.declare_dram_parameter("wp", [128, NCE, E], B16, isOutput=False)
    bq_d = nc.declare_dram_parameter("bq", [128], F32, isOutput=False)
    bv_d = nc.declare_dram_parameter("bv", [PW], F32, isOutput=False)
    bp_d = nc.declare_dram_parameter("bp", [E], F32, isOutput=False)
    gain_d = nc.declare_dram_parameter("gain", [E], F32, isOutput=False)
    beta_d = nc.declare_dram_parameter("beta", [E], F32, isOutput=False)
    out_d = nc.declare_dram_parameter("out", [512, E], F32, isOutput=True)

    a2a_in = nc.dram_tensor("a2a_in", [NCORES, 128, 512], B16)
    a2a_out = nc.dram_tensor("a2a_out", [NCORES, 128, 512], B16)
    # DRAM scratch for the rowsum-reciprocal broadcast bounce
    rs_dram = nc.dram_tensor("rs_scratch", [NSB, 1024], F32)
    rs2_dram = nc.dram_tensor("rs2_scratch", [NSB, 1024], F32)

    with tile.TileContext(nc) as tc:
        from contextlib import ExitStack

        with ExitStack() as ctx:
            consts = ctx.enter_context(tc.tile_pool(name="consts", bufs=1))
            big = ctx.enter_context(tc.tile_pool(name="big", bufs=1))
            epool = ctx.enter_context(tc.tile_pool(name="epool", bufs=3))
            small = ctx.enter_context(tc.tile_pool(name="small", bufs=2))
            yraw = ctx.enter_context(tc.tile_pool(name="yraw", bufs=2))
            ybuf = ctx.enter_context(tc.tile_pool(name="ybuf", bufs=2))
            bcpool = ctx.enter_context(tc.tile_pool(name="bcpool", bufs=2))
            zpool = ctx.enter_context(tc.tile_pool(name="zpool", bufs=2))
            # PSUM: psA = 4 x [128,512] banks (phase-1 accum, attention
            # yA/yB); psB = 2 x [128,1024] (scores double-buffer, phase-3 z)
            psA = ctx.enter_context(tc.tile_pool(name="psA", bufs=4, space="PSUM"))
            psB = ctx.enter_context(tc.tile_pool(name="psB", bufs=2, space="PSUM"))

            xT_ap = xT[:].rearrange("(c p) s -> p c s", p=128)

            # ---- loads, in dependency-urgency order ----
            bq_sb = consts.tile([128, 1], F32)
            nc.sync.dma_start(out=bq_sb,
                              in_=bq_d[:].rearrange("(p u) -> p u", p=128))
            wk_sb = consts.tile([128, NCE, 128], B16)
            nc.sync.dma_start(out=wk_sb, in_=wk_d[:])
            wq_sb = consts.tile([128, NCE, 128], B16)
            nc.sync.dma_start(out=wq_sb, in_=wq_d[:])
            wv_sb = consts.tile([128, NCE, 128], B16)
            nc.scalar.dma_start(out=wv_sb, in_=wv_d[:])

            xTb = big.tile([128, NCE, T], B16)
            for sb in range(NSB):
                sl = slice(sb * 512, (sb + 1) * 512)
                (nc.sync if sb % 2 == 0 else nc.scalar).dma_start(
                    out=xTb[:, :, sl], in_=xT_ap[:, :, sl])

            bv_bc = consts.tile([128, PW], F32)
            nc.gpsimd.dma_start(out=bv_bc, in_=_bcast_ap(bv_d, PW))
            bp_bc = consts.tile([128, E], F32)
            nc.gpsimd.dma_start(out=bp_bc, in_=_bcast_ap(bp_d, E))
            gain_bc = consts.tile([128, E], F32)
            nc.gpsimd.dma_start(out=gain_bc, in_=_bcast_ap(gain_d, E))
            beta_bc = consts.tile([128, E], F32)
            nc.gpsimd.dma_start(out=beta_bc, in_=_bcast_ap(beta_d, E))
            wp_sb = big.tile([128, NCE, E], B16)
            nc.gpsimd.dma_start(out=wp_sb, in_=wp_d[:])

            # ---- phase 1a: K/Q projections into [d, token] layout ----
            # (bk is skipped entirely: softmax is invariant to per-query
            # constant shifts, and q . bk is constant across keys)
            kT = big.tile([128, T], B16)
            qT = big.tile([128, T], B16)
            for sb in range(NSB):
                sl = slice(sb * 512, (sb + 1) * 512)
                ps = psA.tile([128, 512], F32, tag="ps")
                for c in range(NCE):
                    nc.tensor.matmul(ps, wk_sb[:, c, :], xTb[:, c, sl],
                                     start=(c == 0), stop=(c == NCE - 1))
                nc.vector.tensor_copy(out=kT[:, sl], in_=ps)
                ps2 = psA.tile([128, 512], F32, tag="ps")
                for c in range(NCE):
                    nc.tensor.matmul(ps2, wq_sb[:, c, :], xTb[:, c, sl],
                                     start=(c == 0), stop=(c == NCE - 1))
                nc.vector.tensor_scalar_add(out=qT[:, sl], in0=ps2,
                                            scalar1=bq_sb[:, 0:1])

            # ---- phase 1b: V in [key, hA|ones|hB|ones|pad] layout ----
            v_sb = big.tile([128, NKC, VROW], B16)
            nc.vector.memset(v_sb[:, :, 64:65], 1.0)
            nc.vector.memset(v_sb[:, :, 129:130], 1.0)
            nc.vector.memset(v_sb[:, :, 130:VROW], 0.0)
            for kc4 in range(NKC // 4):
                ps = psA.tile([128, 512], F32, tag="ps")
                for j in range(4):
                    kc = kc4 * 4 + j
                    ksl = slice(kc * 128, (kc + 1) * 128)
                    for c in range(NCE):
                        nc.tensor.matmul(ps[:, j * 128:(j + 1) * 128],
                                         xTb[:, c, ksl], wv_sb[:, c, :],
                                         start=(c == 0), stop=(c == NCE - 1))
                ps4 = ps.rearrange("p (j f) -> p j f", f=128)
                nc.vector.tensor_add(
                    out=v_sb[:, kc4 * 4:(kc4 + 1) * 4, 0:64],
                    in0=ps4[:, :, 0:64], in1=_rep_ap(bv_bc[:, 0:64], 4))
                nc.vector.tensor_add(
                    out=v_sb[:, kc4 * 4:(kc4 + 1) * 4, 65:129],
                    in0=ps4[:, :, 64:128], in1=_rep_ap(bv_bc[:, 64:128], 4))

            # ---- phase 2: attention, one (batch, query-block) at a time ----
            for blk in range(NSB):
                b, qb = divmod(blk, 4)
                qsl = slice(blk * 512, (blk + 1) * 512)
                yA = psA.tile([128, 512], F32, tag="ps")
                yB = psA.tile([128, 512], F32, tag="ps")
                for kc in range(S // 128):
                    gkc = b * (S // 128) + kc
                    ksl = slice(gkc * 128, (gkc + 1) * 128)
                    sc = psB.tile([128, 1024], F32, tag="sc")
                    nc.tensor.matmul(sc[:, 0:512], kT[0:64, ksl],
                                     qT[0:64, qsl], start=True, stop=True,
                                     tile_position=(0, 0))
                    nc.tensor.matmul(sc[:, 512:1024], kT[64:128, ksl],
                                     qT[64:128, qsl], start=True, stop=True,
                                     tile_position=(64, 0))
                    e1 = epool.tile([128, 1024], B16, tag="eT")
                    nc.scalar.activation(out=e1, in_=sc, func=AF.Exp,
                                         scale=1.0 / np.sqrt(D))
                    st, sp = (kc == 0), (kc == S // 128 - 1)
                    nc.tensor.matmul(yA, v_sb[:, gkc, 0:128],
                                     e1[:, 0:512], start=st, stop=sp)
                    nc.tensor.matmul(yB, v_sb[:, gkc, 65:193],
                                     e1[:, 512:1024], start=st, stop=sp)
                # drain psum; row 64 of each copy is the head's rowsum
                yr1 = yraw.tile([VW, 512], F32, tag="yr1")
                nc.vector.tensor_copy(out=yr1, in_=yA[0:VW, :])
                yr2 = yraw.tile([VW, 512], F32, tag="yr2")
                nc.vector.tensor_copy(out=yr2, in_=yB[0:VW, :])
                # bounce rowsums to DRAM, reciprocal in a [128,8]
                # partition-major tile, bounce back, broadcast-load
                nc.sync.dma_start(
                    out=rs_dram[blk, 0:512].rearrange("(u s) -> u s", u=1),
                    in_=yr1[D:VW, :])
                nc.sync.dma_start(
                    out=rs_dram[blk, 512:1024].rearrange("(u s) -> u s", u=1),
                    in_=yr2[D:VW, :])
                rpm = small.tile([128, 8], F32, tag="rpm")
                nc.sync.dma_start(
                    out=rpm, in_=rs_dram[blk, :].rearrange("(u j) -> u j", j=8))
                nc.vector.reciprocal(out=rpm, in_=rpm)
                nc.sync.dma_start(
                    out=rs2_dram[blk, :].rearrange("(u j) -> u j", j=8),
                    in_=rpm)
                y_blk = ybuf.tile([128, 512], B16, tag="yb")
                for j in range(2):
                    bc = bcpool.tile([64, 512], F32, tag=f"bc{j}")
                    apj = rs2_dram[blk, j * 512:(j + 1) * 512]
                    nc.sync.dma_start(out=bc, in_=bass.AP(
                        tensor=apj.tensor, offset=apj.offset,
                        ap=[[0, 64], [1, 512]]))
                    yr = yr1 if j == 0 else yr2
                    nc.vector.tensor_mul(
                        out=y_blk[64 * j:64 * (j + 1), :],
                        in0=yr[0:D, :], in1=bc)
                nc.gpsimd.dma_start(out=a2a_in[blk], in_=y_blk)

            # ---- all-to-all: swap (pair, token-window) -> (window, pair) ----
            nc.gpsimd.collective_compute(
                "AllToAll",
                mybir.AluOpType.bypass,
                replica_groups=[list(range(NCORES))],
                ins=[a2a_in[:]],
                outs=[a2a_out[:]],
            )
            recv = big.tile([128, NCORES, 512], B16)
            nc.sync.dma_start(out=recv,
                              in_=a2a_out[:].rearrange("n p s -> p n s"))

            # ---- phase 3: output projection + layernorm ----
            for qb in range(4):
                z = psB.tile([128, 1024], F32, tag="sc")
                for half in range(2):
                    hsl = slice(half * 512, (half + 1) * 512)
                    for i in range(NCORES):
                        nc.tensor.matmul(z[:, hsl],
                                         recv[:, i, qb * 128:(qb + 1) * 128],
                                         wp_sb[:, i, hsl],
                                         start=(i == 0), stop=(i == NCORES - 1))
                zs = zpool.tile([128, E], F32, tag="zs")
                nc.vector.tensor_add(out=zs, in0=z, in1=bp_bc)
                st = small.tile([128, 2, 6], F32, tag="st")
                nc.vector.bn_stats(out=st[:, 0, :], in_=zs[:, 0:512])
                nc.vector.bn_stats(out=st[:, 1, :], in_=zs[:, 512:1024])
                mv = small.tile([128, 2], F32, tag="mv")
                nc.vector.bn_aggr(out=mv, in_=st)
                # reference: (x - mean) / (std + eps), std with ddof=1
                std = small.tile([128, 1], F32, tag="std")
                nc.scalar.activation(out=std, in_=mv[:, 1:2], func=AF.Sqrt,
                                     scale=float(E) / float(E - 1))
                nc.vector.tensor_scalar_add(out=std, in0=std, scalar1=1e-6)
                rinv = small.tile([128, 1], F32, tag="rinv")
                nc.vector.reciprocal(out=rinv, in_=std)
                nc.vector.tensor_scalar(out=zs, in0=zs, scalar1=mv[:, 0:1],
                                        scalar2=rinv, op0=OP.subtract,
                                        op1=OP.mult)
                nc.vector.tensor_mul(out=zs, in0=zs, in1=gain_bc)
                nc.vector.tensor_add(out=zs, in0=zs, in1=beta_bc)
                nc.sync.dma_start(out=out_d[qb * 128:(qb + 1) * 128, :], in_=zs)

    _split_drain_waits(nc)
    return nc


def _get_program():
    if "nc" not in _CACHE:
        _CACHE["nc"] = _build_program()
    return _CACHE["nc"]


def _make_in_maps(inputs):
    x = np.ascontiguousarray(np.asarray(inputs["x"], dtype=np.float32))
    w = {k: np.asarray(inputs[k], np.float32).astype(BF16)
         for k in ("Wq", "Wk", "Wv", "Wp")}
    vecs = {k: np.ascontiguousarray(np.asarray(inputs[k], np.float32))
            for k in ("bq", "bv", "bp", "gain", "beta")}

    # [E, T] with both batches' tokens concatenated, bf16
    xT = np.ascontiguousarray(
        np.concatenate([x[0].T, x[1].T], axis=1).astype(BF16))

    def shuf_cols(W, c):  # [E, E] -> core c's [128, NCE, 128] slice
        return np.ascontiguousarray(
            W[:, c * 128:(c + 1) * 128].reshape(NCE, 128, 128)
            .transpose(1, 0, 2))

    wp = np.ascontiguousarray(
        w["Wp"].reshape(NCE, 128, E).transpose(1, 0, 2))

    in_maps = []
    for core in range(NCORES):
        in_maps.append({
            "xT": xT,
            "wq": shuf_cols(w["Wq"], core),
            "wk": shuf_cols(w["Wk"], core),
            "wv": shuf_cols(w["Wv"], core),
            "wp": wp,
            "bq": np.ascontiguousarray(
                vecs["bq"][core * 128:(core + 1) * 128]),
            "bv": np.ascontiguousarray(
                vecs["bv"][core * 128:(core + 1) * 128]),
            "bp": vecs["bp"], "gain": vecs["gain"], "beta": vecs["beta"],
        })
    return in_maps


def _assemble(results):
    full = np.empty((B, S, E), dtype=np.float32)
    for core in range(NCORES):
        b, qs = divmod(core, NCORES // B)
        full[b, qs * 512:(qs + 1) * 512, :] = results[core]["out"]
    return full


def kernel(**inputs):
    nc = _get_program()
    in_maps = _make_in_maps(inputs)
    res = run_bass_kernel_spmd(nc, in_maps, core_ids=list(range(NCORES)))
    return _assemble(res.results)


def _ensure_ntff_hook():
    """The agent image's antenv lacks axon_hooks; synthesize it so that
    run_bass_kernel_spmd(trace=True) can fetch NTFF profiles via the
    libaxon_pjrt.so ctypes path that trn_agent_boot already ships."""
    import sys
    import types

    try:
        from antenv.axon_hooks import get_axon_ntff_profile_hook  # noqa: F401
        return
    except ImportError:
        pass
    from trn_agent_boot.trn_boot import _ntff_profile_via_ctypes

    mod = types.ModuleType("antenv.axon_hooks")
    state = {"hook": None}
    mod.set_axon_ntff_profile_hook = lambda h: state.__setitem__("hook", h)
    mod.get_axon_ntff_profile_hook = lambda: state["hook"]
    sys.modules["antenv.axon_hooks"] = mod
    import antenv

    antenv.axon_hooks = mod
    mod.set_axon_ntff_profile_hook(
        _ntff_profile_via_ctypes("/opt/axon/libaxon_pjrt.so"))


def run_traced(inputs, trace_cores=None):
    """Used by test.py: returns (full_output, BassKernelResults with timing)."""
    _ensure_ntff_hook()
    nc = _get_program()
    in_maps = _make_in_maps(inputs)
    res = run_bass_kernel_spmd(nc, in_maps, core_ids=list(range(NCORES)),
                               trace=True, trace_cores=trace_cores)
    return _assemble(res.results), res


# revision 7
# speedup vs baseline: 1.1408x; 1.1408x over previous
"""Multi-head attention + layernorm Bass kernel for Trainium2 (8 NeuronCores).

Sharding: tensor-parallel over heads. Core c owns global heads (2c, 2c+1)
for BOTH batches: it computes Q/K/V projections for its 128 feature
columns, attention for its head pair over all 4096 tokens, then an 8-way
AllToAll exchanges attention outputs so core c ends up holding all 1024
features for token window c (batch c//4, queries (c%4)*512..+512), where it
runs the output projection + layernorm.

Schedule: phase 2 is Act-engine bound (exp), so all projection work that
batch-0 attention doesn't need (kT/qT/V for batch 1, qT for later batch-0
query blocks) is deferred and woven into phase 2's per-key-chunk PE slack
via a deadline-paced thunk queue. Last attention block's rowsum chain uses
a single merged psum tile so the pre-A2A latency chain is short, and dummy
matmuls keep the PE HAM-warm across the AllToAll wait.

Layouts:
  - xT [E, T=4096] bf16 (both batches' tokens), loaded per 512-token block
  - kT/qT in [d, token] (128 partitions = 2 heads x 64 d)
  - scores TRANSPOSED per key chunk: sc[key, qA|qB] via two quadrant
    matmuls; exp output feeds AV directly
  - V in [key, hA 0:64 |ones| hB 65:129 |ones| pad] so each AV matmul
    also emits that head's softmax rowsum in psum row 64
  - yAB psum [128, qA 0:512 | qB 512:1024]; rows 64 of each half = rowsums
  - softmax skips max-subtraction: |scores|/8 <= ~9 for this distribution
  - AllToAll buffers [8, 128, 512] bf16: shard j = my pair's normalized y
    for token window j; received slot i = pair i's y for my window
"""

import numpy as np
import ml_dtypes

import concourse.bass as bass
import concourse.mybir as mybir
import concourse.tile as tile
from concourse.bass_utils import run_bass_kernel_spmd

BF16 = ml_dtypes.bfloat16
F32 = mybir.dt.float32
B16 = mybir.dt.bfloat16

B, S, E, H, D = 2, 2048, 1024, 16, 64
T = B * S         # 4096 tokens across both batches
NCORES = 8
NCE = E // 128    # 8 contraction chunks over E
NSB = T // 512    # 8 token blocks (also the 8 attention blocks / A2A shards)
NKC = T // 128    # 32 key chunks
SKC = S // 128    # 16 key chunks per batch
PW = 2 * D        # 128 features per head pair
VW = D + 1        # 65: head's V columns + ones column
VROW = 200        # vA[0:64] ones[64] vB[65:129] ones[129] pad[130:200]

_CACHE = {}


def _bcast_ap(handle, n):
    """AP reading a [n]-element DRAM vector broadcast across 128 partitions."""
    ap = handle[:]
    return bass.AP(tensor=ap.tensor, offset=ap.offset, ap=[[0, 128], [1, n]])


def _rep_ap(sbuf_slice, rep):
    """Repeat a [128, n] SBUF slice `rep` times along a middle axis."""
    ap = sbuf_slice
    return bass.AP(tensor=ap.tensor, offset=ap.offset,
                   ap=[ap.ap[0], [0, rep], ap.ap[1]])


def _split_drain_waits(nc):
    """This walrus build encodes at most ONE sem wait per instruction;
    Tile emits several on some (drain, multi-dep compute/DMA). Merge waits
    on the same semaphore (sem-ge-imm: max value implies the rest), then
    hoist all but the last onto standalone EventSemaphore instructions
    placed just before, in the same engine's stream."""
    n = 0
    for f in nc.m.functions:
        for blk in f.blocks:
            new_insts = []
            for inst in blk.instructions:
                si = getattr(inst, "sync_info", None)
                if si is not None and len(si.on_wait) > 1:
                    merged = {}
                    rest = []
                    for w in si.on_wait:
                        if w.wait_mode == "sem-ge-imm":
                            k = w.id
                            if k not in merged or merged[k].wait_value < w.wait_value:
                                merged[k] = w
                        else:
                            rest.append(w)
                    waits = rest + list(merged.values())
                    for w in waits[:-1]:
                        n += 1
                        ev = mybir.InstEventSemaphore(
                            name=f"I-splitwait-{n}",
                            ins=[], outs=[],
                            sync_info=mybir.SyncInfo(on_wait=[w], on_update=[]),
                        )
                        ev.engine = inst.engine
                        new_insts.append(ev)
                    inst.sync_info = mybir.SyncInfo(
                        on_wait=[waits[-1]], on_update=list(si.on_update))
                new_insts.append(inst)
            blk.instructions[:] = new_insts
    return n


def _build_program():
    nc = bass.Bass()
    AF = mybir.ActivationFunctionType
    OP = mybir.AluOpType

    xT = nc.declare_dram_parameter("xT", [E, T], B16, isOutput=False)
    # per-core 128-column slices of Wq/Wk/Wv, host-shuffled to [p, c, d]
    wq_d = nc.declare_dram_parameter("wq", [128, NCE, 128], B16, isOutput=False)
    wk_d = nc.declare_dram_parameter("wk", [128, NCE, 128], B16, isOutput=False)
    wv_d = nc.declare_dram_parameter("wv", [128, NCE, 128], B16, isOutput=False)
    wp_d = nc.declare_dram_parameter("wp", [128, NCE, E], B16, isOutput=False)
    bq_d = nc.declare_dram_parameter("bq", [128], F32, isOutput=False)
    bv_d = nc.declare_dram_parameter("bv", [PW], F32, isOutput=False)
    bp_d = nc.declare_dram_parameter("bp", [E], F32, isOutput=False)
    gain_d = nc.declare_dram_parameter("gain", [E], F32, isOutput=False)
    beta_d = nc.declare_dram_parameter("beta", [E], F32, isOutput=False)
    out_d = nc.declare_dram_parameter("out", [512, E], F32, isOutput=True)

    a2a_in = nc.dram_tensor("a2a_in", [NCORES, 128, 512], B16)
    a2a_out = nc.dram_tensor("a2a_out", [NCORES, 128, 512], B16)
    # DRAM scratch for the rowsum-reciprocal broadcast bounce
    rs_dram = nc.dram_tensor("rs_scratch", [NSB, 1024], F32)
    rs2_dram = nc.dram_tensor("rs2_scratch", [NSB, 1024], F32)

    with tile.TileContext(nc) as tc:
        from contextlib import ExitStack

        with ExitStack() as ctx:
            consts = ctx.enter_context(tc.tile_pool(name="consts", bufs=1))
            big = ctx.enter_context(tc.tile_pool(name="big", bufs=1))
            epool = ctx.enter_context(tc.tile_pool(name="epool", bufs=3))
            small = ctx.enter_context(tc.tile_pool(name="small", bufs=2))
            yraw = ctx.enter_context(tc.tile_pool(name="yraw", bufs=2))
            ybuf = ctx.enter_context(tc.tile_pool(name="ybuf", bufs=2))
            bcpool = ctx.enter_context(tc.tile_pool(name="bcpool", bufs=2))
            zpool = ctx.enter_context(tc.tile_pool(name="zpool", bufs=2))
            # PSUM: psA 2x[128,512] (projection accums, warm dummies),
            # psc 2x[128,1024] (scores / phase-3 z), psy 1x[128,1024] (yAB)
            psA = ctx.enter_context(tc.tile_pool(name="psA", bufs=2, space="PSUM"))
            psc = ctx.enter_context(tc.tile_pool(name="psc", bufs=2, space="PSUM"))
            psy = ctx.enter_context(tc.tile_pool(name="psy", bufs=1, space="PSUM"))

            xT_ap = xT[:].rearrange("(c p) s -> p c s", p=128)

            # ---- loads: batch-0 critical path first ----
            bq_sb = consts.tile([128, 1], F32)
            nc.sync.dma_start(out=bq_sb,
                              in_=bq_d[:].rearrange("(p u) -> p u", p=128))
            wk_sb = consts.tile([128, NCE, 128], B16)
            nc.sync.dma_start(out=wk_sb, in_=wk_d[:])
            wq_sb = consts.tile([128, NCE, 128], B16)
            nc.sync.dma_start(out=wq_sb, in_=wq_d[:])
            wv_sb = consts.tile([128, NCE, 128], B16)
            nc.scalar.dma_start(out=wv_sb, in_=wv_d[:])

            xTb = big.tile([128, NCE, T], B16)
            for sb in range(NSB):
                sl = slice(sb * 512, (sb + 1) * 512)
                (nc.sync if sb % 2 == 0 else nc.scalar).dma_start(
                    out=xTb[:, :, sl], in_=xT_ap[:, :, sl])

            bv_bc = consts.tile([128, PW], F32)
            nc.gpsimd.dma_start(out=bv_bc, in_=_bcast_ap(bv_d, PW))
            bp_bc = consts.tile([128, E], F32)
            nc.gpsimd.dma_start(out=bp_bc, in_=_bcast_ap(bp_d, E))
            gain_bc = consts.tile([128, E], F32)
            nc.gpsimd.dma_start(out=gain_bc, in_=_bcast_ap(gain_d, E))
            beta_bc = consts.tile([128, E], F32)
            nc.gpsimd.dma_start(out=beta_bc, in_=_bcast_ap(beta_d, E))
            wp_sb = big.tile([128, NCE, E], B16)
            nc.gpsimd.dma_start(out=wp_sb, in_=wp_d[:])

            kT = big.tile([128, T], B16)
            qT = big.tile([128, T], B16)
            v_sb = big.tile([128, NKC, VROW], B16)
            nc.vector.memset(v_sb[:, :, 64:65], 1.0)
            nc.vector.memset(v_sb[:, :, 129:130], 1.0)
            nc.vector.memset(v_sb[:, :, 130:VROW], 0.0)

            # ---- projection emitters (phase 1 now, or deferred thunks) ----
            def kq_thunks(w_sb, sb, is_q):
                sl = slice(sb * 512, (sb + 1) * 512)
                st = {}

                def mk(c):
                    def f():
                        if c == 0:
                            st["ps"] = psA.tile([128, 512], F32, tag="ps", name="dps")
                        nc.tensor.matmul(st["ps"], w_sb[:, c, :],
                                         xTb[:, c, sl],
                                         start=(c == 0), stop=(c == NCE - 1))
                        if c == NCE - 1:
                            if is_q:
                                nc.vector.tensor_scalar_add(
                                    out=qT[:, sl], in0=st["ps"],
                                    scalar1=bq_sb[:, 0:1])
                            else:
                                nc.vector.tensor_copy(out=kT[:, sl],
                                                      in_=st["ps"])
                    return f
                return [mk(c) for c in range(NCE)]

            def v_thunks(kc4):
                st = {}

                def mk(j, c):
                    def f():
                        if j == 0 and c == 0:
                            st["ps"] = psA.tile([128, 512], F32, tag="ps", name="dps")
                        kc = kc4 * 4 + j
                        ksl = slice(kc * 128, (kc + 1) * 128)
                        nc.tensor.matmul(st["ps"][:, j * 128:(j + 1) * 128],
                                         xTb[:, c, ksl], wv_sb[:, c, :],
                                         start=(c == 0), stop=(c == NCE - 1))
                        if j == 3 and c == NCE - 1:
                            ps4 = st["ps"].rearrange("p (u f) -> p u f", f=128)
                            nc.vector.tensor_add(
                                out=v_sb[:, kc4 * 4:(kc4 + 1) * 4, 0:64],
                                in0=ps4[:, :, 0:64],
                                in1=_rep_ap(bv_bc[:, 0:64], 4))
                            nc.vector.tensor_add(
                                out=v_sb[:, kc4 * 4:(kc4 + 1) * 4, 65:129],
                                in0=ps4[:, :, 64:128],
                                in1=_rep_ap(bv_bc[:, 64:128], 4))
                    return f
                return [mk(j, c) for j in range(4) for c in range(NCE)]

            # ---- phase 1 (minimal): only what batch-0 attention needs ----
            for th in kq_thunks(wq_sb, 0, True):
                th()
            for sb in range(4):
                for th in kq_thunks(wk_sb, sb, False):
                    th()
            for kc4 in range(4):
                for th in v_thunks(kc4):
                    th()

            # deferred: (deadline_block, thunk, est_cost_ns)
            deferred = []
            for sb in (1, 2, 3):
                deferred += [(sb, th, 220) for th in kq_thunks(wq_sb, sb, True)]
            deferred += [(4, th, 220) for th in kq_thunks(wq_sb, 4, True)]
            for sb in (4, 5, 6, 7):
                deferred += [(4, th, 220) for th in kq_thunks(wk_sb, sb, False)]
            for kc4 in (4, 5, 6, 7):
                deferred += [(4, th, 70) for th in v_thunks(kc4)]
            for sb in (5, 6, 7):
                deferred += [(sb, th, 220) for th in kq_thunks(wq_sb, sb, True)]
            SLOT_NS = 380

            # ---- phase 2: attention, one (batch, query-block) at a time ----
            for blk in range(NSB):
                b, qb = divmod(blk, 4)
                qsl = slice(blk * 512, (blk + 1) * 512)
                # force-emit anything due before this block
                while deferred and deferred[0][0] <= blk:
                    deferred.pop(0)[1]()
                yAB = psy.tile([128, 1024], F32, tag="y")
                for kc in range(SKC):
                    gkc = b * SKC + kc
                    ksl = slice(gkc * 128, (gkc + 1) * 128)
                    sc = psc.tile([128, 1024], F32, tag="sc")
                    nc.tensor.matmul(sc[:, 0:512], kT[0:64, ksl],
                                     qT[0:64, qsl], start=True, stop=True,
                                     tile_position=(0, 0))
                    nc.tensor.matmul(sc[:, 512:1024], kT[64:128, ksl],
                                     qT[64:128, qsl], start=True, stop=True,
                                     tile_position=(64, 0))
                    e1 = epool.tile([128, 1024], B16, tag="eT")
                    nc.scalar.activation(out=e1, in_=sc, func=AF.Exp,
                                         scale=1.0 / np.sqrt(D))
                    st, sp = (kc == 0), (kc == SKC - 1)
                    nc.tensor.matmul(yAB[:, 0:512], v_sb[:, gkc, 0:128],
                                     e1[:, 0:512], start=st, stop=sp)
                    nc.tensor.matmul(yAB[:, 512:1024], v_sb[:, gkc, 65:193],
                                     e1[:, 512:1024], start=st, stop=sp)
                    # weave deferred projection work into the Act-bound slack
                    budget = SLOT_NS
                    while deferred and budget > 0:
                        _, th, cost = deferred.pop(0)
                        th()
                        budget -= cost
                # drain psum; row 64 of each half is that head's rowsum
                yr = yraw.tile([VW, 1024], F32, tag="yr")
                nc.vector.tensor_copy(out=yr, in_=yAB[0:VW, :])
                nc.sync.dma_start(
                    out=rs_dram[blk, :].rearrange("(u s) -> u s", u=1),
                    in_=yr[D:VW, :])
                rpm = small.tile([128, 8], F32, tag="rpm")
                nc.sync.dma_start(
                    out=rpm, in_=rs_dram[blk, :].rearrange("(u j) -> u j", j=8))
                nc.vector.reciprocal(out=rpm, in_=rpm)
                nc.sync.dma_start(
                    out=rs2_dram[blk, :].rearrange("(u j) -> u j", j=8),
                    in_=rpm)
                y_blk = ybuf.tile([128, 512], B16, tag="yb")
                for j in range(2):
                    bc = bcpool.tile([64, 512], F32, tag=f"bc{j}")
                    apj = rs2_dram[blk, j * 512:(j + 1) * 512]
                    (nc.sync if j == 0 else nc.gpsimd).dma_start(
                        out=bc, in_=bass.AP(
                            tensor=apj.tensor, offset=apj.offset,
                            ap=[[0, 64], [1, 512]]))
                    nc.vector.tensor_mul(
                        out=y_blk[64 * j:64 * (j + 1), :],
                        in0=yr[0:D, j * 512:(j + 1) * 512], in1=bc)
                nc.gpsimd.dma_start(out=a2a_in[blk], in_=y_blk)

            # ---- all-to-all: swap (pair, token-window) -> (window, pair) ----
            nc.gpsimd.collective_compute(
                "AllToAll",
                mybir.AluOpType.bypass,
                replica_groups=[list(range(NCORES))],
                ins=[a2a_in[:]],
                outs=[a2a_out[:]],
            )
            # keep the PE HAM-warm across the collective wait (outputs unread)
            for w in range(24):
                warm = psA.tile([128, 512], F32, tag="ps")
                nc.tensor.matmul(warm, wv_sb[:, 0, :], xTb[:, 0, 0:512],
                                 start=True, stop=True)
            recv = big.tile([128, NCORES, 512], B16)
            nc.sync.dma_start(out=recv,
                              in_=a2a_out[:].rearrange("n p s -> p n s"))

            # ---- phase 3: output projection + layernorm ----
            for qb in range(4):
                z = psc.tile([128, 1024], F32, tag="sc")
                for half in range(2):
                    hsl = slice(half * 512, (half + 1) * 512)
                    for i in range(NCORES):
                        nc.tensor.matmul(z[:, hsl],
                                         recv[:, i, qb * 128:(qb + 1) * 128],
                                         wp_sb[:, i, hsl],
                                         start=(i == 0), stop=(i == NCORES - 1))
                zs = zpool.tile([128, E], F32, tag="zs")
                nc.vector.tensor_add(out=zs, in0=z, in1=bp_bc)
                st = small.tile([128, 2, 6], F32, tag="st")
                nc.vector.bn_stats(out=st[:, 0, :], in_=zs[:, 0:512])
                nc.vector.bn_stats(out=st[:, 1, :], in_=zs[:, 512:1024])
                mv = small.tile([128, 2], F32, tag="mv")
                nc.vector.bn_aggr(out=mv, in_=st)
                # reference: (x - mean) / (std + eps), std with ddof=1
                std = small.tile([128, 1], F32, tag="std")
                nc.scalar.activation(out=std, in_=mv[:, 1:2], func=AF.Sqrt,
                                     scale=float(E) / float(E - 1))
                nc.vector.tensor_scalar_add(out=std, in0=std, scalar1=1e-6)
                rinv = small.tile([128, 1], F32, tag="rinv")
                nc.vector.reciprocal(out=rinv, in_=std)
                nmr = small.tile([128, 1], F32, tag="nmr")
                nc.vector.tensor_scalar(out=nmr, in0=mv[:, 0:1], scalar1=rinv,
                                        scalar2=-1.0, op0=OP.mult, op1=OP.mult)
                zn = zpool.tile([128, E], F32, tag="zn")
                nc.scalar.activation(out=zn, in_=zs, func=AF.Identity,
                                     scale=rinv[:, 0:1], bias=nmr[:, 0:1])
                nc.gpsimd.tensor_mul(out=zn, in0=zn, in1=gain_bc)
                nc.gpsimd.tensor_add(out=zn, in0=zn, in1=beta_bc)
                nc.sync.dma_start(out=out_d[qb * 128:(qb + 1) * 128, :], in_=zn)

    _split_drain_waits(nc)
    return nc


def _get_program():
    if "nc" not in _CACHE:
        _CACHE["nc"] = _build_program()
    return _CACHE["nc"]


def _make_in_maps(inputs):
    x = np.ascontiguousarray(np.asarray(inputs["x"], dtype=np.float32))
    w = {k: np.asarray(inputs[k], np.float32).astype(BF16)
         for k in ("Wq", "Wk", "Wv", "Wp")}
    vecs = {k: np.ascontiguousarray(np.asarray(inputs[k], np.float32))
            for k in ("bq", "bv", "bp", "gain", "beta")}

    # [E, T] with both batches' tokens concatenated, bf16
    xT = np.ascontiguousarray(
        np.concatenate([x[0].T, x[1].T], axis=1).astype(BF16))

    def shuf_cols(W, c):  # [E, E] -> core c's [128, NCE, 128] slice
        return np.ascontiguousarray(
            W[:, c * 128:(c + 1) * 128].reshape(NCE, 128, 128)
            .transpose(1, 0, 2))

    wp = np.ascontiguousarray(
        w["Wp"].reshape(NCE, 128, E).transpose(1, 0, 2))

    in_maps = []
    for core in range(NCORES):
        in_maps.append({
            "xT": xT,
            "wq": shuf_cols(w["Wq"], core),
            "wk": shuf_cols(w["Wk"], core),
            "wv": shuf_cols(w["Wv"], core),
            "wp": wp,
            "bq": np.ascontiguousarray(
                vecs["bq"][core * 128:(core + 1) * 128]),
            "bv": np.ascontiguousarray(
                vecs["bv"][core * 128:(core + 1) * 128]),
            "bp": vecs["bp"], "gain": vecs["gain"], "beta": vecs["beta"],
        })
    return in_maps


def _assemble(results):
    full = np.empty((B, S, E), dtype=np.float32)
    for core in range(NCORES):
        b, qs = divmod(core, NCORES // B)
        full[b, qs * 512:(qs + 1) * 512, :] = results[core]["out"]
    return full


def kernel(**inputs):
    nc = _get_program()
    in_maps = _make_in_maps(inputs)
    res = run_bass_kernel_spmd(nc, in_maps, core_ids=list(range(NCORES)))
    return _assemble(res.results)


def _ensure_ntff_hook():
    """The agent image's antenv lacks axon_hooks; synthesize it so that
    run_bass_kernel_spmd(trace=True) can fetch NTFF profiles via the
    libaxon_pjrt.so ctypes path that trn_agent_boot already ships."""
    import sys
    import types

    try:
        from antenv.axon_hooks import get_axon_ntff_profile_hook  # noqa: F401
        return
    except ImportError:
        pass
    from trn_agent_boot.trn_boot import _ntff_profile_via_ctypes

    mod = types.ModuleType("antenv.axon_hooks")
    state = {"hook": None}
    mod.set_axon_ntff_profile_hook = lambda h: state.__setitem__("hook", h)
    mod.get_axon_ntff_profile_hook = lambda: state["hook"]
    sys.modules["antenv.axon_hooks"] = mod
    import antenv

    antenv.axon_hooks = mod
    mod.set_axon_ntff_profile_hook(
        _ntff_profile_via_ctypes("/opt/axon/libaxon_pjrt.so"))


def run_traced(inputs, trace_cores=None):
    """Used by test.py: returns (full_output, BassKernelResults with timing)."""
    _ensure_ntff_hook()
    nc = _get_program()
    in_maps = _make_in_maps(inputs)
    res = run_bass_kernel_spmd(nc, in_maps, core_ids=list(range(NCORES)),
                               trace=True, trace_cores=trace_cores)
    return _assemble(res.results), res


# revision 8
# speedup vs baseline: 1.1509x; 1.0088x over previous
"""Multi-head attention + layernorm Bass kernel for Trainium2 (8 NeuronCores).

Sharding: tensor-parallel over heads. Core c owns global heads (2c, 2c+1)
for BOTH batches: it computes Q/K/V projections for its 128 feature
columns, attention for its head pair over all 4096 tokens, then an 8-way
AllToAll exchanges attention outputs so core c ends up holding all 1024
features for token window c (batch c//4, queries (c%4)*512..+512), where it
runs the output projection + layernorm.

Schedule: phase 2 is Act-engine bound (exp), so all projection work that
batch-0 attention doesn't need (kT/qT/V for batch 1, qT for later batch-0
query blocks) is deferred and woven into phase 2's per-key-chunk PE slack
via a deadline-paced thunk queue. Last attention block's rowsum chain uses
a single merged psum tile so the pre-A2A latency chain is short, and dummy
matmuls keep the PE HAM-warm across the AllToAll wait.

Layouts:
  - xT [E, T=4096] bf16 (both batches' tokens), loaded per 512-token block
  - kT/qT in [d, token] (128 partitions = 2 heads x 64 d)
  - scores TRANSPOSED per key chunk: sc[key, qA|qB] via two quadrant
    matmuls; exp output feeds AV directly
  - V in [key, hA 0:64 |ones| hB 65:129 |ones| pad] so each AV matmul
    also emits that head's softmax rowsum in psum row 64
  - yAB psum [128, qA 0:512 | qB 512:1024]; rows 64 of each half = rowsums
  - softmax skips max-subtraction: |scores|/8 <= ~9 for this distribution
  - AllToAll buffers [8, 128, 512] bf16: shard j = my pair's normalized y
    for token window j; received slot i = pair i's y for my window
"""

import numpy as np
import ml_dtypes

import concourse.bass as bass
import concourse.mybir as mybir
import concourse.tile as tile
from concourse.bass_utils import run_bass_kernel_spmd

BF16 = ml_dtypes.bfloat16
F32 = mybir.dt.float32
B16 = mybir.dt.bfloat16

B, S, E, H, D = 2, 2048, 1024, 16, 64
T = B * S         # 4096 tokens across both batches
NCORES = 8
NCE = E // 128    # 8 contraction chunks over E
NSB = T // 512    # 8 token blocks (also the 8 attention blocks / A2A shards)
NKC = T // 128    # 32 key chunks
SKC = S // 128    # 16 key chunks per batch
PW = 2 * D        # 128 features per head pair
VW = D + 1        # 65: head's V columns + ones column
VROW = 200        # vA[0:64] ones[64] vB[65:129] ones[129] pad[130:200]

_CACHE = {}


def _bcast_ap(handle, n):
    """AP reading a [n]-element DRAM vector broadcast across 128 partitions."""
    ap = handle[:]
    return bass.AP(tensor=ap.tensor, offset=ap.offset, ap=[[0, 128], [1, n]])


def _rep_ap(sbuf_slice, rep):
    """Repeat a [128, n] SBUF slice `rep` times along a middle axis."""
    ap = sbuf_slice
    return bass.AP(tensor=ap.tensor, offset=ap.offset,
                   ap=[ap.ap[0], [0, rep], ap.ap[1]])


def _split_drain_waits(nc):
    """This walrus build encodes at most ONE sem wait per instruction;
    Tile emits several on some (drain, multi-dep compute/DMA). Merge waits
    on the same semaphore (sem-ge-imm: max value implies the rest), then
    hoist all but the last onto standalone EventSemaphore instructions
    placed just before, in the same engine's stream."""
    n = 0
    for f in nc.m.functions:
        for blk in f.blocks:
            new_insts = []
            for inst in blk.instructions:
                si = getattr(inst, "sync_info", None)
                if si is not None and len(si.on_wait) > 1:
                    merged = {}
                    rest = []
                    for w in si.on_wait:
                        if w.wait_mode == "sem-ge-imm":
                            k = w.id
                            if k not in merged or merged[k].wait_value < w.wait_value:
                                merged[k] = w
                        else:
                            rest.append(w)
                    waits = rest + list(merged.values())
                    for w in waits[:-1]:
                        n += 1
                        ev = mybir.InstEventSemaphore(
                            name=f"I-splitwait-{n}",
                            ins=[], outs=[],
                            sync_info=mybir.SyncInfo(on_wait=[w], on_update=[]),
                        )
                        ev.engine = inst.engine
                        new_insts.append(ev)
                    inst.sync_info = mybir.SyncInfo(
                        on_wait=[waits[-1]], on_update=list(si.on_update))
                new_insts.append(inst)
            blk.instructions[:] = new_insts
    return n


def _build_program():
    nc = bass.Bass()
    AF = mybir.ActivationFunctionType
    OP = mybir.AluOpType

    xT_d = nc.declare_dram_parameter("xT", [NSB, 128, NCE, 512], B16,
                                   isOutput=False)
    # per-core 128-column slices of Wq/Wk/Wv, host-shuffled to [p, c, d]
    wq_d = nc.declare_dram_parameter("wq", [128, NCE, 128], B16, isOutput=False)
    wk_d = nc.declare_dram_parameter("wk", [128, NCE, 128], B16, isOutput=False)
    wv_d = nc.declare_dram_parameter("wv", [128, NCE, 128], B16, isOutput=False)
    wp_d = nc.declare_dram_parameter("wp", [128, NCE, E], B16, isOutput=False)
    bq_d = nc.declare_dram_parameter("bq", [128], F32, isOutput=False)
    bv_d = nc.declare_dram_parameter("bv", [PW], F32, isOutput=False)
    bp_d = nc.declare_dram_parameter("bp", [E], F32, isOutput=False)
    gain_d = nc.declare_dram_parameter("gain", [E], F32, isOutput=False)
    beta_d = nc.declare_dram_parameter("beta", [E], F32, isOutput=False)
    out_d = nc.declare_dram_parameter("out", [512, E], F32, isOutput=True)

    warm_in = nc.dram_tensor("warm_in", [NCORES, 64], F32)
    warm_out = nc.dram_tensor("warm_out", [NCORES, 64], F32)
    a2a_in = nc.dram_tensor("a2a_in", [NCORES, 128, 512], B16)
    a2a_out = nc.dram_tensor("a2a_out", [NCORES, 128, 512], B16)
    # DRAM scratch for the rowsum-reciprocal broadcast bounce
    rs_dram = nc.dram_tensor("rs_scratch", [NSB, 1024], F32)
    rs2_dram = nc.dram_tensor("rs2_scratch", [NSB, 1024], F32)

    with tile.TileContext(nc) as tc:
        from contextlib import ExitStack

        with ExitStack() as ctx:
            consts = ctx.enter_context(tc.tile_pool(name="consts", bufs=1))
            big = ctx.enter_context(tc.tile_pool(name="big", bufs=1))
            epool = ctx.enter_context(tc.tile_pool(name="epool", bufs=3))
            small = ctx.enter_context(tc.tile_pool(name="small", bufs=2))
            yraw = ctx.enter_context(tc.tile_pool(name="yraw", bufs=2))
            ybuf = ctx.enter_context(tc.tile_pool(name="ybuf", bufs=2))
            bcpool = ctx.enter_context(tc.tile_pool(name="bcpool", bufs=2))
            zpool = ctx.enter_context(tc.tile_pool(name="zpool", bufs=2))
            # PSUM: psA 2x[128,512] (projection accums, warm dummies),
            # psc 2x[128,1024] (scores / phase-3 z), psy 1x[128,1024] (yAB)
            psA = ctx.enter_context(tc.tile_pool(name="psA", bufs=2, space="PSUM"))
            psc = ctx.enter_context(tc.tile_pool(name="psc", bufs=2, space="PSUM"))
            psy = ctx.enter_context(tc.tile_pool(name="psy", bufs=1, space="PSUM"))

            # ---- loads: batch-0 critical path first ----
            bq_sb = consts.tile([128, 1], F32)
            nc.sync.dma_start(out=bq_sb,
                              in_=bq_d[:].rearrange("(p u) -> p u", p=128))
            wk_sb = consts.tile([128, NCE, 128], B16)
            nc.sync.dma_start(out=wk_sb, in_=wk_d[:])
            wq_sb = consts.tile([128, NCE, 128], B16)
            nc.sync.dma_start(out=wq_sb, in_=wq_d[:])
            wv_sb = consts.tile([128, NCE, 128], B16)
            nc.scalar.dma_start(out=wv_sb, in_=wv_d[:])

            nc.gpsimd.collective_compute(
                "AllToAll",
                mybir.AluOpType.bypass,
                replica_groups=[list(range(NCORES))],
                ins=[warm_in[:]],
                outs=[warm_out[:]],
            )
            xTb = big.tile([128, NCE, T], B16)
            for sb in range(NSB):
                sl = slice(sb * 512, (sb + 1) * 512)
                (nc.sync if sb % 2 == 0 else nc.scalar).dma_start(
                    out=xTb[:, :, sl], in_=xT_d[sb])

            bv_bc = consts.tile([128, PW], F32)
            nc.gpsimd.dma_start(out=bv_bc, in_=_bcast_ap(bv_d, PW))
            bp_bc = consts.tile([128, E], F32)
            nc.gpsimd.dma_start(out=bp_bc, in_=_bcast_ap(bp_d, E))
            gain_bc = consts.tile([128, E], F32)
            nc.gpsimd.dma_start(out=gain_bc, in_=_bcast_ap(gain_d, E))
            beta_bc = consts.tile([128, E], F32)
            nc.gpsimd.dma_start(out=beta_bc, in_=_bcast_ap(beta_d, E))
            wp_sb = big.tile([128, NCE, E], B16)
            nc.gpsimd.dma_start(out=wp_sb, in_=wp_d[:])

            kT = big.tile([128, T], B16)
            qT = big.tile([128, T], B16)
            v_sb = big.tile([128, NKC, VROW], B16)
            nc.vector.memset(v_sb[:, :, 64:65], 1.0)
            nc.vector.memset(v_sb[:, :, 129:130], 1.0)
            nc.vector.memset(v_sb[:, :, 130:VROW], 0.0)

            # ---- projection emitters (phase 1 now, or deferred thunks) ----
            def kq_thunks(w_sb, sb, is_q):
                sl = slice(sb * 512, (sb + 1) * 512)
                st = {}

                def mk(c):
                    def f():
                        if c == 0:
                            st["ps"] = psA.tile([128, 512], F32, tag="ps", name="dps")
                        nc.tensor.matmul(st["ps"], w_sb[:, c, :],
                                         xTb[:, c, sl],
                                         start=(c == 0), stop=(c == NCE - 1))
                        if c == NCE - 1:
                            if is_q:
                                nc.vector.tensor_scalar_add(
                                    out=qT[:, sl], in0=st["ps"],
                                    scalar1=bq_sb[:, 0:1])
                            else:
                                nc.vector.tensor_copy(out=kT[:, sl],
                                                      in_=st["ps"])
                    return f
                return [mk(c) for c in range(NCE)]

            def v_thunks(kc4):
                st = {}

                def mk(j, c):
                    def f():
                        if j == 0 and c == 0:
                            st["ps"] = psA.tile([128, 512], F32, tag="ps", name="dps")
                        kc = kc4 * 4 + j
                        ksl = slice(kc * 128, (kc + 1) * 128)
                        nc.tensor.matmul(st["ps"][:, j * 128:(j + 1) * 128],
                                         xTb[:, c, ksl], wv_sb[:, c, :],
                                         start=(c == 0), stop=(c == NCE - 1))
                        if j == 3 and c == NCE - 1:
                            ps4 = st["ps"].rearrange("p (u f) -> p u f", f=128)
                            nc.vector.tensor_add(
                                out=v_sb[:, kc4 * 4:(kc4 + 1) * 4, 0:64],
                                in0=ps4[:, :, 0:64],
                                in1=_rep_ap(bv_bc[:, 0:64], 4))
                            nc.vector.tensor_add(
                                out=v_sb[:, kc4 * 4:(kc4 + 1) * 4, 65:129],
                                in0=ps4[:, :, 64:128],
                                in1=_rep_ap(bv_bc[:, 64:128], 4))
                    return f
                return [mk(j, c) for j in range(4) for c in range(NCE)]

            # ---- phase 1 (minimal): only what batch-0 attention needs ----
            for th in kq_thunks(wq_sb, 0, True):
                th()
            for sb in range(4):
                for th in kq_thunks(wk_sb, sb, False):
                    th()
            for kc4 in range(4):
                for th in v_thunks(kc4):
                    th()

            # deferred: (deadline_block, thunk, est_cost_ns)
            deferred = []
            for sb in (1, 2, 3):
                deferred += [(sb, th, 220) for th in kq_thunks(wq_sb, sb, True)]
            deferred += [(4, th, 220) for th in kq_thunks(wq_sb, 4, True)]
            for sb in (4, 5, 6, 7):
                deferred += [(4, th, 220) for th in kq_thunks(wk_sb, sb, False)]
            for kc4 in (4, 5, 6, 7):
                deferred += [(4, th, 70) for th in v_thunks(kc4)]
            for sb in (5, 6, 7):
                deferred += [(sb, th, 220) for th in kq_thunks(wq_sb, sb, True)]
            SLOT_NS = 320

            # ---- phase 2: attention, one (batch, query-block) at a time ----
            for blk in range(NSB):
                b, qb = divmod(blk, 4)
                qsl = slice(blk * 512, (blk + 1) * 512)
                # force-emit anything due before this block
                while deferred and deferred[0][0] <= blk:
                    deferred.pop(0)[1]()
                yAB = psy.tile([128, 1024], F32, tag="y")
                for kc in range(SKC):
                    gkc = b * SKC + kc
                    ksl = slice(gkc * 128, (gkc + 1) * 128)
                    sc = psc.tile([128, 1024], F32, tag="sc")
                    nc.tensor.matmul(sc[:, 0:512], kT[0:64, ksl],
                                     qT[0:64, qsl], start=True, stop=True,
                                     tile_position=(0, 0))
                    nc.tensor.matmul(sc[:, 512:1024], kT[64:128, ksl],
                                     qT[64:128, qsl], start=True, stop=True,
                                     tile_position=(64, 0))
                    e1 = epool.tile([128, 1024], B16, tag="eT")
                    nc.scalar.activation(out=e1, in_=sc, func=AF.Exp,
                                         scale=1.0 / np.sqrt(D))
                    st, sp = (kc == 0), (kc == SKC - 1)
                    nc.tensor.matmul(yAB[:, 0:512], v_sb[:, gkc, 0:128],
                                     e1[:, 0:512], start=st, stop=sp)
                    nc.tensor.matmul(yAB[:, 512:1024], v_sb[:, gkc, 65:193],
                                     e1[:, 512:1024], start=st, stop=sp)
                    # weave deferred projection work into the Act-bound slack
                    budget = SLOT_NS
                    while deferred and budget > 0:
                        _, th, cost = deferred.pop(0)
                        th()
                        budget -= cost
                # drain psum; row 64 of each half is that head's rowsum
                yr = yraw.tile([VW, 1024], F32, tag="yr")
                nc.vector.tensor_copy(out=yr, in_=yAB[0:VW, :])
                nc.sync.dma_start(
                    out=rs_dram[blk, :].rearrange("(u s) -> u s", u=1),
                    in_=yr[D:VW, :])
                rpm = small.tile([128, 8], F32, tag="rpm")
                nc.sync.dma_start(
                    out=rpm, in_=rs_dram[blk, :].rearrange("(u j) -> u j", j=8))
                nc.vector.reciprocal(out=rpm, in_=rpm)
                nc.sync.dma_start(
                    out=rs2_dram[blk, :].rearrange("(u j) -> u j", j=8),
                    in_=rpm)
                y_blk = ybuf.tile([128, 512], B16, tag="yb")
                for j in range(2):
                    bc = bcpool.tile([64, 512], F32, tag=f"bc{j}")
                    apj = rs2_dram[blk, j * 512:(j + 1) * 512]
                    (nc.sync if j == 0 else nc.gpsimd).dma_start(
                        out=bc, in_=bass.AP(
                            tensor=apj.tensor, offset=apj.offset,
                            ap=[[0, 64], [1, 512]]))
                    nc.vector.tensor_mul(
                        out=y_blk[64 * j:64 * (j + 1), :],
                        in0=yr[0:D, j * 512:(j + 1) * 512], in1=bc)
                    nc.gpsimd.dma_start(
                        out=a2a_in[blk, 64 * j:64 * (j + 1), :],
                        in_=y_blk[64 * j:64 * (j + 1), :])

            # ---- all-to-all: swap (pair, token-window) -> (window, pair) ----
            nc.gpsimd.collective_compute(
                "AllToAll",
                mybir.AluOpType.bypass,
                replica_groups=[list(range(NCORES))],
                ins=[a2a_in[:]],
                outs=[a2a_out[:]],
            )
            # keep the PE HAM-warm across the collective wait (outputs unread)
            for w in range(24):
                warm = psA.tile([128, 512], F32, tag="ps")
                nc.tensor.matmul(warm, wv_sb[:, 0, :], xTb[:, 0, 0:512],
                                 start=True, stop=True)
            recv = big.tile([128, NCORES, 512], B16)
            nc.sync.dma_start(
                out=recv[:, 0:4, :],
                in_=a2a_out[0:4].rearrange("n p s -> p n s"))
            nc.scalar.dma_start(
                out=recv[:, 4:8, :],
                in_=a2a_out[4:8].rearrange("n p s -> p n s"))

            # ---- phase 3: output projection + layernorm ----
            for qb in range(4):
                z = psc.tile([128, 1024], F32, tag="sc")
                for half in range(2):
                    hsl = slice(half * 512, (half + 1) * 512)
                    for i in range(NCORES):
                        nc.tensor.matmul(z[:, hsl],
                                         recv[:, i, qb * 128:(qb + 1) * 128],
                                         wp_sb[:, i, hsl],
                                         start=(i == 0), stop=(i == NCORES - 1))
                zs = zpool.tile([128, E], F32, tag="zs")
                nc.vector.tensor_add(out=zs, in0=z, in1=bp_bc)
                st = small.tile([128, 2, 6], F32, tag="st")
                nc.vector.bn_stats(out=st[:, 0, :], in_=zs[:, 0:512])
                nc.vector.bn_stats(out=st[:, 1, :], in_=zs[:, 512:1024])
                mv = small.tile([128, 2], F32, tag="mv")
                nc.vector.bn_aggr(out=mv, in_=st)
                # reference: (x - mean) / (std + eps), std with ddof=1
                std = small.tile([128, 1], F32, tag="std")
                nc.scalar.activation(out=std, in_=mv[:, 1:2], func=AF.Sqrt,
                                     scale=float(E) / float(E - 1))
                nc.vector.tensor_scalar_add(out=std, in0=std, scalar1=1e-6)
                rinv = small.tile([128, 1], F32, tag="rinv")
                nc.vector.reciprocal(out=rinv, in_=std)
                nmr = small.tile([128, 1], F32, tag="nmr")
                nc.vector.tensor_scalar(out=nmr, in0=mv[:, 0:1], scalar1=rinv,
                                        scalar2=-1.0, op0=OP.mult, op1=OP.mult)
                zn = zpool.tile([128, E], F32, tag="zn")
                nc.scalar.activation(out=zn, in_=zs, func=AF.Identity,
                                     scale=rinv[:, 0:1], bias=nmr[:, 0:1])
                nc.gpsimd.tensor_mul(out=zn, in0=zn, in1=gain_bc)
                nc.gpsimd.tensor_add(out=zn, in0=zn, in1=beta_bc)
                nc.sync.dma_start(out=out_d[qb * 128:(qb + 1) * 128, :], in_=zn)

    _split_drain_waits(nc)
    return nc


def _get_program():
    if "nc" not in _CACHE:
        _CACHE["nc"] = _build_program()
    return _CACHE["nc"]


def _make_in_maps(inputs):
    x = np.ascontiguousarray(np.asarray(inputs["x"], dtype=np.float32))
    w = {k: np.asarray(inputs[k], np.float32).astype(BF16)
         for k in ("Wq", "Wk", "Wv", "Wp")}
    vecs = {k: np.ascontiguousarray(np.asarray(inputs[k], np.float32))
            for k in ("bq", "bv", "bp", "gain", "beta")}

    # both batches' tokens concatenated, re-tiled so each 512-token
    # block is one DMA with contiguous 8KB-per-partition reads
    xTf = np.concatenate([x[0].T, x[1].T], axis=1).astype(BF16)
    xT = np.ascontiguousarray(
        xTf.reshape(NCE, 128, NSB, 512).transpose(2, 1, 0, 3))

    def shuf_cols(W, c):  # [E, E] -> core c's [128, NCE, 128] slice
        return np.ascontiguousarray(
            W[:, c * 128:(c + 1) * 128].reshape(NCE, 128, 128)
            .transpose(1, 0, 2))

    wp = np.ascontiguousarray(
        w["Wp"].reshape(NCE, 128, E).transpose(1, 0, 2))

    in_maps = []
    for core in range(NCORES):
        in_maps.append({
            "xT": xT,
            "wq": shuf_cols(w["Wq"], core),
            "wk": shuf_cols(w["Wk"], core),
            "wv": shuf_cols(w["Wv"], core),
            "wp": wp,
            "bq": np.ascontiguousarray(
                vecs["bq"][core * 128:(core + 1) * 128]),
            "bv": np.ascontiguousarray(
                vecs["bv"][core * 128:(core + 1) * 128]),
            "bp": vecs["bp"], "gain": vecs["gain"], "beta": vecs["beta"],
        })
    return in_maps


def _assemble(results):
    full = np.empty((B, S, E), dtype=np.float32)
    for core in range(NCORES):
        b, qs = divmod(core, NCORES // B)
        full[b, qs * 512:(qs + 1) * 512, :] = results[core]["out"]
    return full


def kernel(**inputs):
    nc = _get_program()
    in_maps = _make_in_maps(inputs)
    res = run_bass_kernel_spmd(nc, in_maps, core_ids=list(range(NCORES)))
    return _assemble(res.results)


def _ensure_ntff_hook():
    """The agent image's antenv lacks axon_hooks; synthesize it so that
    run_bass_kernel_spmd(trace=True) can fetch NTFF profiles via the
    libaxon_pjrt.so ctypes path that trn_agent_boot already ships."""
    import sys
    import types

    try:
        from antenv.axon_hooks import get_axon_ntff_profile_hook  # noqa: F401
        return
    except ImportError:
        pass
    from trn_agent_boot.trn_boot import _ntff_profile_via_ctypes

    mod = types.ModuleType("antenv.axon_hooks")
    state = {"hook": None}
    mod.set_axon_ntff_profile_hook = lambda h: state.__setitem__("hook", h)
    mod.get_axon_ntff_profile_hook = lambda: state["hook"]
    sys.modules["antenv.axon_hooks"] = mod
    import antenv

    antenv.axon_hooks = mod
    mod.set_axon_ntff_profile_hook(
        _ntff_profile_via_ctypes("/opt/axon/libaxon_pjrt.so"))


def run_traced(inputs, trace_cores=None):
    """Used by test.py: returns (full_output, BassKernelResults with timing)."""
    _ensure_ntff_hook()
    nc = _get_program()
    in_maps = _make_in_maps(inputs)
    res = run_bass_kernel_spmd(nc, in_maps, core_ids=list(range(NCORES)),
                               trace=True, trace_cores=trace_cores)
    return _assemble(res.results), res
